# revision 23
# baseline (speedup 1.0000x reference)
"""Causal multi-head attention kernel for Trainium2 (8 NeuronCores), v3.

Problem: x[1,2048,1024] -> qkv proj (W_qkv[1024,3072]) -> 64 heads of dim 16
         -> causal softmax attention -> out proj (W_out[1024,1024]).

Sharding: Megatron-style head parallelism. Each of the 8 cores owns 8 heads
(a 128-wide column slice of W_qkv per q/k/v and a 128-row slice of W_out),
computes a partial output projection, and the host sums the 8 partials
(the "all-reduce").

v12 (~200us) changes vs v2 (316us baseline):
  * Host pre-transposes x into the [chan, chunk, token] bf16 layout and
    pre-spaces/pre-casts every weight to its SBUF layout: the entire 38us
    on-device staging prologue (fp32 loads + DVE casts + xbar transposes)
    becomes ~5MB of straight DMAs; matmuls start at ~3us.
  * PV matmuls are emitted one tile LATE (software-pipeline lag 1): the PE
    queue head never waits on an in-flight exp, so the PE stays dense and
    the HAM clock gate holds 2.4 GHz (v2 spent 189us at 1.2 GHz).
  * exp runs on BOTH ScalarE (table exp) and VectorE (Schraudolph bf16
    bit-trick, ~3% per-element error) with a tunable per-tile pattern,
    including diagonal tiles.
  * One full-width [128,512] pv*recip write per (group, qn) replaces four
    16-row strips; the softmax identity rowsum*recip==1 lands 1.0 in the
    spare partition rows, which doubles as the b_out bias row via an extra
    row in the pre-spaced W_out (bias comes free out of the out-proj MM).
  * Outproj evacuations alternate ACT/DVE; memsets moved to GpSimd; y is
    written bf16 (host sums partials in fp32).
  * outproj is deferred into the exp-bound late-qn phases (outproj(0)->qn2,
    outproj(1,2)->qn3 capped per group, outproj(3)->tail on the freed psco
    ring) so the PE never starves where the HAM clock gate would re-throttle.

  * HAM clock-gate feeding (the big one, ~227us -> ~200us): the HAM busy
    metric tracks PE array *switching activity*, not instruction occupancy —
    phases of pure S (16 live rows) / PV (17 live cols) matmuls re-throttle
    the PE to 1.2 GHz even with a dense instruction stream.  Fixes: q/k are
    duplicated across both 16-row halves of each 32-row group at 1/sqrt(2)
    (host prep) so S contracts 32 dense rows for the identical score; V's 15
    zero-pad columns carry duplicated v values (GpSimd copy) so PV/outproj
    run dense (the extra attnT rows meet zero W_out rows); dense qkv/v/
    outproj units are spread 1-per-2-tiles through the attention stream.

Known failed experiments (do not retry blindly): single-head [128,512] score
tiles with psco=4/ppv=2/pfil=2 (335us - instruction/sem count dominates);
all-ACT outproj evacs (278us - ACT head-of-line blocking); V computed as VT
with DMA-xbar transpose into the strided V layout (NaN - dma_start_transpose
requires the dst last dim to be the full in-partition extent); concentrating
outproj(1,2) into qn=3 (202us - qn=2 loses warmth; the even 1-per-2-tile
spread with outproj(s)->qn=s+1 is better).

Self-contained: hardcodes all shapes; host code only reshapes/slices inputs
per core and sums the 8 partial outputs.
"""

import numpy as np
from contextlib import ExitStack

import ml_dtypes

import concourse.bass as bass
import concourse.tile as tile
from concourse import mybir
from concourse.bass_utils import run_bass_kernel_spmd

F32 = mybir.dt.float32
BF16 = mybir.dt.bfloat16
I16 = mybir.dt.int16
AF = mybir.ActivationFunctionType

T = 2048
C = 1024
HDIM = 16
NHEADS = 64
NCORES = 8
HPC = NHEADS // NCORES      # 8 heads per core
CSLICE = HPC * HDIM         # 128 channel slice per core
G = 2                       # head groups of 4 per core
NCH = C // 128              # 8 contraction chunks
NT = T // 128               # 16 token chunks of 128
NQ = T // 512               # 4 query blocks of 512

# Schraudolph bf16 exp2: bits = round(EXPQ_MUL * s + EXPQ_ADD) viewed as bf16
# approximates exp(0.25*s).  128*log2(e)*0.25 = 46.166...; 16256 = 127<<7.
EXPQ_MUL = 128.0 * 0.25 * 1.4426950408889634
EXPQ_ADD = 16256.0 - 5.5

# Per-tile exp engine within each attention group: A=ScalarE exp table,
# D=VectorE Schraudolph trick.  Tile 0 of each group is pattern[0]; keep it
# 'A' so the first tile isn't stuck behind the previous group's DVE work.
EXP_PATTERN = "AD"
# PSUM evacuation engine alternation for qk/outproj units.
EVAC_PATTERN = "AD"

_CACHE = {}


def _legalize_waits(nc):
    """This neuronxcc/walrus build encodes at most ONE sync-wait per
    instruction (two on EventSemaphore) — multi-wait sync_info dies in
    codegen with "Too many sync wait commands".  Hoist excess waits into
    standalone EventSemaphore instructions on the same engine immediately
    before the instruction (engine queues are in-order, so semantics are
    preserved)."""
    import bass_rust
    n = 0
    for f in nc.m.functions:
        for blk in f.blocks:
            out = []
            changed = False
            for inst in blk.instructions:
                si = inst.sync_info
                waits = list(si.on_wait) if si is not None and si.on_wait else []
                cap = 2 if isinstance(inst, mybir.InstEventSemaphore) else 1
                if len(waits) > cap:
                    extra, keep = waits[:-cap], waits[-cap:]
                    for i in range(0, len(extra), 2):
                        ev = mybir.InstEventSemaphore(
                            name=f"evwait-{n}", ins=[], outs=[])
                        n += 1
                        ev.engine = inst.engine
                        ev.sync_info = bass_rust.SyncInfo(
                            on_wait=extra[i:i + 2], on_update=[])
                        out.append(ev)
                    inst.sync_info = bass_rust.SyncInfo(
                        on_wait=keep,
                        on_update=list(si.on_update) if si.on_update else [])
                    changed = True
                out.append(inst)
            if changed:
                blk.instructions = out
    return n


def _build_nc():
    nc = bass.Bass()

    xt_d = nc.declare_dram_parameter("xt", [128, NCH * T], BF16, isOutput=False)
    wq_d = nc.declare_dram_parameter("wq", [128, G * NCH * 128], BF16, isOutput=False)
    wk_d = nc.declare_dram_parameter("wk", [128, G * NCH * 128], BF16, isOutput=False)
    wv_d = nc.declare_dram_parameter("wv", [128, NCH * CSLICE], BF16, isOutput=False)
    wo_d = nc.declare_dram_parameter("wo", [128, G * C], BF16, isOutput=False)
    bq_d = nc.declare_dram_parameter("bq", [G, 128], F32, isOutput=False)
    bk_d = nc.declare_dram_parameter("bk", [G, 128], F32, isOutput=False)
    bv_d = nc.declare_dram_parameter("bv", [1, CSLICE], F32, isOutput=False)
    tri_d = nc.declare_dram_parameter("tri", [128, 128], BF16, isOutput=False)
    y_d = nc.declare_dram_parameter("y", [T, C], BF16, isOutput=True)

    with tile.TileContext(nc) as tc, ExitStack() as ctx:
        consts = ctx.enter_context(tc.tile_pool(name="consts", bufs=1))
        stage = ctx.enter_context(tc.tile_pool(name="stage", bufs=3))
        epool = ctx.enter_context(tc.tile_pool(name="epool", bufs=6))
        small = ctx.enter_context(tc.tile_pool(name="small", bufs=2))

        psco = ctx.enter_context(tc.tile_pool(name="psco", bufs=3, space="PSUM"))
        ppv = ctx.enter_context(tc.tile_pool(name="ppv", bufs=1, space="PSUM"))
        pfil = ctx.enter_context(tc.tile_pool(name="pfil", bufs=1, space="PSUM"))

        # ---- persistent tiles ----
        xT = consts.tile([128, NCH, T], BF16)   # xT[c, cc, t] = x[t, 128cc+c]
        qT = consts.tile([128, G, T], BF16)     # spaced: head j at part 32j
        kT = consts.tile([128, G, T], BF16)
        V = consts.tile([128, NT, HPC * 32], BF16)  # [t, tt, 8*32]: 16 dims +
        # rowsum-ones col + zero pad per head (packed PV writes 32 rows/head)
        vr = V.rearrange("p t (h e) -> p t h e", h=HPC)
        attnT = consts.tile([128, G, T], BF16)  # full 128 rows written by the
        # normalize TT: head rows = p/rowsum, spare rows = {1.0 (bias), 0}

        wq_sb = consts.tile([128, G, NCH, 128], BF16)
        wk_sb = consts.tile([128, G, NCH, 128], BF16)
        wv_sb = consts.tile([128, NCH, CSLICE], BF16)
        wo_sb = consts.tile([128, G, C], BF16)
        tri = consts.tile([128, 128], BF16)
        eps_sb = consts.tile([128, 1], F32)
        bq_sb = consts.tile([128, G], F32)
        bk_sb = consts.tile([128, G], F32)
        bv_sb = consts.tile([128, CSLICE], F32)

        # ---- input DMAs, two queues in parallel, first-needed first ----
        # sync queue: activations; scalar queue: weights/consts (ScalarE is
        # idle until the first exp anyway).
        xt_r = xt_d.rearrange("p (q a t) -> p q a t", q=NQ, a=NCH)
        nc.sync.dma_start(out=xT[:, 0:4, 0:512], in_=xt_r[:, 0, 0:4])
        nc.sync.dma_start(out=xT[:, 4:8, 0:512], in_=xt_r[:, 0, 4:8])
        wq_r = wq_d.rearrange("p (g a w) -> p g a w", g=G, a=NCH)
        wk_r = wk_d.rearrange("p (g a w) -> p g a w", g=G, a=NCH)
        nc.scalar.dma_start(out=wq_sb[:, 0], in_=wq_r[:, 0])
        nc.scalar.dma_start(out=wk_sb[:, 0], in_=wk_r[:, 0])
        nc.scalar.dma_start(out=tri, in_=tri_d[:, :])
        nc.scalar.dma_start(out=bq_sb, in_=bq_d.rearrange("g p -> p g"))
        nc.scalar.dma_start(out=bk_sb, in_=bk_d.rearrange("g p -> p g"))
        nc.scalar.dma_start(out=wq_sb[:, 1], in_=wq_r[:, 1])
        nc.scalar.dma_start(out=wk_sb[:, 1], in_=wk_r[:, 1])
        nc.scalar.dma_start(out=wv_sb, in_=wv_d.rearrange(
            "p (a w) -> p a w", a=NCH))
        nc.scalar.dma_start(out=bv_sb, in_=bv_d[0:1, :].to_broadcast((128, CSLICE)))
        nc.scalar.dma_start(out=wo_sb, in_=wo_d.rearrange("p (g w) -> p g w", g=G))

        nc.vector.memset(eps_sb, 1e-30)
        # V pad columns: zeros at 17..31, rowsum-ones at 16 (GpSimd is idle)
        nc.gpsimd.memset(vr[:, :, :, HDIM:32], 0.0)
        nc.gpsimd.memset(vr[:, :, :, HDIM:HDIM + 1], 1.0)

        # ---- engine alternation counters ----
        exp_idx = [0]
        evac_idx = [0]

        def next_evac():
            e = EVAC_PATTERN[evac_idx[0] % len(EVAC_PATTERN)]
            evac_idx[0] += 1
            return e

        # ---- pipeline building blocks ----
        qk_open = {}

        def qk_half(g, qn, which, half):
            """One 4-chunk half of a q or k projection accumulation.
            half=0 opens the PSUM group; half=1 finishes it + bias evac."""
            w_sb, b_sb, dst = ((wq_sb, bq_sb, qT) if which == "q"
                               else (wk_sb, bk_sb, kT))
            if half == 0:
                qk_open[(which, g)] = pfil.tile(
                    [128, 512], F32, tag="fil", name="filps")
            ps_t = qk_open[(which, g)]
            for i in range(4):
                cc = 4 * half + i
                nc.tensor.matmul(
                    out=ps_t, lhsT=w_sb[:, g, cc, :],
                    rhs=xT[:, cc, qn * 512:(qn + 1) * 512],
                    start=(cc == 0), stop=(cc == NCH - 1),
                )
            if half == 1:
                del qk_open[(which, g)]
                nc.vector.tensor_scalar_add(
                    out=dst[:, g, qn * 512:(qn + 1) * 512], in0=ps_t,
                    scalar1=b_sb[:, g:g + 1])

        def v_tile(tt):
            ps_t = pfil.tile([128, 512], F32, tag="fil", name="vps")
            ps = ps_t[:, 0:CSLICE]
            for cc in range(NCH):
                nc.tensor.matmul(
                    out=ps, lhsT=xT[:, cc, tt * 128:(tt + 1) * 128],
                    rhs=wv_sb[:, cc, :],
                    start=(cc == 0), stop=(cc == NCH - 1),
                )
            nc.vector.tensor_tensor(
                vr[:, tt, :, 0:HDIM], ps.rearrange("p (h e) -> p h e", h=HPC),
                bv_sb.rearrange("p (h e) -> p h e", h=HPC),
                mybir.AluOpType.add,
            )
            # duplicate v into the 15 pad columns (GpSimd, off the critical
            # engines): PV then produces finite garbage in attnT rows whose
            # W_out rows are zero — results unchanged, but the PV and
            # outproj matmuls run with ~2x the PE array switching activity,
            # which keeps the HAM clock-gate busy metric fed.
            nc.gpsimd.tensor_copy(vr[:, tt, :, HDIM + 1:32],
                                  vr[:, tt, :, 1:HDIM])

        def outproj_unit(tt, nn, tail=False):
            if tail:
                # the attention psco ring is free at the tail: 3-deep
                # rotation instead of the single pfil buffer
                ps = psco.tile([128, 1024], F32, tag="sset", name="sset")[:, 0:512]
            else:
                ps = pfil.tile([128, 512], F32, tag="fil", name="ops")
            for g in range(G):
                nc.tensor.matmul(
                    out=ps, lhsT=attnT[:, g, tt * 128:(tt + 1) * 128],
                    rhs=wo_sb[:, g, nn * 512:(nn + 1) * 512],
                    start=(g == 0), stop=(g == G - 1),
                )
            ys = stage.tile([128, 512], BF16, tag="yout", name="ys")
            if next_evac() == "A":
                nc.scalar.activation(out=ys, in_=ps, func=AF.Copy)
            else:
                nc.vector.tensor_copy(ys, ps)
            nc.sync.dma_start(
                out=y_d[tt * 128:(tt + 1) * 128, nn * 512:(nn + 1) * 512],
                in_=ys,
            )

        # filler scheduling: `fillers` (qkv/v — must drain within their qn so
        # consumers see the writes in emission order) popped 1/tile;
        # `late` (outproj — consumers are only the output DMAs) strided
        # across the exp-bound late-qn tile streams to keep the PE dense.
        fillers = []
        late = []

        def pop_any(k):
            for _ in range(k):
                if fillers:
                    fillers.pop(0)()
                elif late:
                    late.pop(0)()
                else:
                    return

        def attn_group(g, qn, late_cap=None):
            """Attention for 4 heads (group g) x 512 queries (block qn).
            Software-pipelined with PV lag 1: per step i we emit exp(i),
            PV(i-1), a filler, S(i+2) — the PE queue head never waits on an
            exp that was issued the same step, so the PE stream stays dense
            (keeps the HAM clock gate at 2.4 GHz)."""
            pv = ppv.tile([128, 512], F32, tag="pv")
            nkc = 4 * qn + 4
            tiles = [(kc, a) for kc in range(nkc) for a in range(2)]
            n = len(tiles)
            ssets = {}
            ets = {}

            def emit_S(i):
                kc, a = tiles[i]
                f0 = max(0, 128 * (kc - 4 * qn))
                sset = psco.tile([128, 1024], F32, tag="sset", name="sset")
                for jj in range(2):
                    j = 2 * a + jj
                    # q/k rows are duplicated across both 16-row halves at
                    # 1/sqrt(2) (host prep), so the 32-row contraction gives
                    # the same score with 2x the PE array switching activity
                    # (keeps the HAM clock gate's busy metric fed).
                    nc.tensor.matmul(
                        out=sset[:, 512 * jj + f0:512 * jj + 512],
                        lhsT=kT[32 * j:32 * j + 32, g, kc * 128:(kc + 1) * 128],
                        rhs=qT[32 * j:32 * j + 32, g, qn * 512 + f0:(qn + 1) * 512],
                        start=True, stop=True,
                        tile_position=(32 * j, 0),
                    )
                ssets[i] = sset

            def emit_exp(i):
                kc, a = tiles[i]
                jjj = kc - 4 * qn          # >=0: diagonal-straddling tile
                f0 = max(0, 128 * jjj)
                sset = ssets.pop(i)
                et = epool.tile([128, 1024], BF16, tag="expT", name="et")
                er = et.rearrange("p (h q) -> p h q", h=2)
                sr = sset.rearrange("p (h q) -> p h q", h=2)
                eng = EXP_PATTERN[exp_idx[0] % len(EXP_PATTERN)]
                exp_idx[0] += 1
                if eng == "D":
                    eb = et.bitcast(I16).rearrange("p (h q) -> p h q", h=2)
                    nc.vector.tensor_scalar(
                        out=eb[:, :, f0:512], in0=sr[:, :, f0:512],
                        scalar1=EXPQ_MUL, scalar2=EXPQ_ADD,
                        op0=mybir.AluOpType.mult,
                        op1=mybir.AluOpType.add,
                    )
                else:
                    nc.scalar.activation(
                        out=er[:, :, f0:512], in_=sr[:, :, f0:512],
                        func=AF.Exp, scale=0.25)
                if jjj >= 0:
                    # triangle-mask the diagonal stripe: on DVE for tricked
                    # tiles (bf16 2x mode, same queue as the trick -> no
                    # cross-engine hop), on GpSimd for ACT tiles.
                    if eng == "D":
                        nc.vector.tensor_tensor(
                            er[:, :, f0:f0 + 128], er[:, :, f0:f0 + 128],
                            tri[:, None, :].to_broadcast((128, 2, 128)),
                            mybir.AluOpType.mult,
                        )
                    else:
                        nc.gpsimd.tensor_tensor(
                            er[:, :, f0:f0 + 128], er[:, :, f0:f0 + 128],
                            tri[:, None, :].to_broadcast((128, 2, 128)),
                            mybir.AluOpType.mult,
                        )
                ets[i] = et

            def emit_PV(i):
                kc, a = tiles[i]
                f0 = max(0, 128 * (kc - 4 * qn))
                et = ets.pop(i)
                for jj in range(2):
                    j = 2 * a + jj
                    h = 4 * g + j
                    nc.tensor.matmul(
                        out=pv[32 * j:32 * j + 32, f0:512],
                        lhsT=V[:, kc, 32 * h:32 * h + 32],
                        rhs=et[:, 512 * jj + f0:512 * jj + 512],
                        start=(kc == 0), stop=(kc == nkc - 1),
                        tile_position=(0, 32 * j),
                        # sim group tracker is partition-base blind;
                        # packed heads write disjoint partitions
                        skip_group_check=True,
                    )

            exp_idx[0] = 0
            # spread the available late (outproj) fillers across this group's
            # tiles once the qkv/v fillers run dry; cap so later groups in
            # the same qn still get their share
            budget = len(late) if late_cap is None else min(late_cap, len(late))
            pops = 0
            emit_S(0)
            emit_S(1)
            for i in range(n):
                emit_exp(i)
                if i >= 1:
                    emit_PV(i - 1)
                if i % 2 == 0:
                    if fillers:
                        fillers.pop(0)()
                    elif late and pops < budget:
                        late.pop(0)()
                        pops += 1
                if i + 2 < n:
                    emit_S(i + 2)
            emit_PV(n - 1)
            # normalize: 1/rowsum via exp(-ln(x+eps)); garbage lanes may go
            # NaN/inf but only the (positive) rowsum lanes are ever read.
            ln_t = small.tile([128, 512], F32, tag="lnt")
            nc.scalar.activation(out=ln_t, in_=pv, func=AF.Ln, bias=eps_sb[:, 0:1])
            rec_t = small.tile([128, 512], F32, tag="rect")
            nc.scalar.activation(out=rec_t, in_=ln_t, func=AF.Exp, scale=-1.0)
            rec_rep = small.tile([128, 512], F32, tag="recrep")
            nc.vector.stream_shuffle(rec_rep, rec_t, [HDIM] * 32)
            # one full-width write: head rows p*recip, rowsum rows 1.0 (the
            # out-proj bias row), V-pad rows 0.
            nc.vector.tensor_tensor(
                attnT[:, g, qn * 512:(qn + 1) * 512], pv, rec_rep,
                mybir.AluOpType.mult,
            )
            # keep the PE fed while ln/exp/shuffle/mult run
            pop_any(3)

        # ---- emission: fused qn-major pipeline ----
        # qkv(qn=0): group 0 + v emitted directly, group 1 rides as filler
        for half in range(2):
            qk_half(0, 0, "q", half)
        for half in range(2):
            qk_half(0, 0, "k", half)
        for tt in range(4):
            v_tile(tt)
        for half in range(2):
            fillers.append(lambda h=half: qk_half(1, 0, "q", h))
        for half in range(2):
            fillers.append(lambda h=half: qk_half(1, 0, "k", h))

        for qn in range(NQ):
            if qn + 1 < NQ:
                # next query block's activations (1MB, ~3us) — issued here so
                # the early sync queue stays clear for the V transposes
                nc.sync.dma_start(out=xT[:, :, (qn + 1) * 512:(qn + 2) * 512],
                                  in_=xt_r[:, qn + 1])
                for g in range(G):
                    for half in range(2):
                        fillers.append(
                            lambda g=g, qn=qn, h=half: qk_half(g, qn + 1, "q", h))
                    for half in range(2):
                        fillers.append(
                            lambda g=g, qn=qn, h=half: qk_half(g, qn + 1, "k", h))
                # v for qn+1 except the last block, whose tiles ride inside
                # qn=3 itself (its diagonal kc 12..15 aren't read until late
                # in each group, so emitting them early in qn=3 is safe and
                # keeps dense MMs flowing there)
                if qn < 2:
                    for tt in range(4 * qn + 4, 4 * qn + 8):
                        fillers.append(lambda tt=tt: v_tile(tt))
            if qn == 3:
                for tt in range(12, 16):
                    fillers.append(lambda tt=tt: v_tile(tt))
            # outproj is deferred one block (dense 128-contraction MMs keep
            # the PE array activity high where pure S/PV tiles would let the
            # HAM clock gate re-throttle): outproj(s) -> qn=s+1, 3 -> tail.
            if qn > 0:
                s = qn - 1
                for tt in range(4 * s, 4 * s + 4):
                    for nn in range(2):
                        late.append(lambda tt=tt, nn=nn: outproj_unit(tt, nn))
            for g in range(G):
                attn_group(g, qn, late_cap=(4 if g == 0 else None))
            # qkv/v fillers must be fully emitted before qn+1 consumes them
            while fillers:
                fillers.pop(0)()
        while late:
            late.pop(0)()
        for tt in range(12, 16):
            for nn in range(2):
                outproj_unit(tt, nn, tail=True)
    return nc


def _make_in_maps(x, W_qkv, b_qkv, W_out, b_out):
    BF = ml_dtypes.bfloat16
    x2 = np.asarray(x, dtype=np.float32).reshape(T, C)
    W_qkv = np.asarray(W_qkv, dtype=np.float32)
    b_qkv = np.asarray(b_qkv, dtype=np.float32)
    W_out = np.asarray(W_out, dtype=np.float32)
    b_out = np.asarray(b_out, dtype=np.float32)

    # chunk-major: xt[c, qn, cc, t'] = x[512qn + t', 128cc + c] so each
    # 512-token chunk DMA reads per-partition-contiguous 8KB runs
    xt = np.ascontiguousarray(
        x2.reshape(NQ, 512, NCH, 128).transpose(3, 0, 2, 1)).astype(BF)
    xt = np.ascontiguousarray(xt.reshape(128, NQ * NCH * 512))

    tri = np.zeros((128, 128), dtype=np.float32)
    for p in range(128):
        tri[p, p:] = 1.0
    tri = tri.astype(BF)

    in_maps = []
    for p in range(NCORES):
        c0 = p * CSLICE

        def spaced_w(cols0):
            # both 16-col halves of each head's 32-col slot carry w/sqrt(2):
            # the kernel contracts scores over the full 32-row group, giving
            # identical results with 2x the PE array switching activity
            blk = W_qkv[:, cols0:cols0 + CSLICE].reshape(NCH, 128, G, 4, HDIM)
            half = blk.transpose(1, 2, 0, 3, 4) * np.float32(1.0 / np.sqrt(2.0))
            sp = np.zeros((128, G, NCH, 4, 32), dtype=np.float32)
            sp[:, :, :, :, :HDIM] = half
            sp[:, :, :, :, HDIM:] = half
            return np.ascontiguousarray(
                sp.reshape(128, G * NCH * 128)).astype(BF)

        wq = spaced_w(c0)
        wk = spaced_w(C + c0)
        # wv[c, cc, e] = W_qkv[128cc+c, 2C + c0 + e]  (dense)
        wv = np.ascontiguousarray(
            W_qkv[:, 2 * C + c0:2 * C + c0 + CSLICE]
            .reshape(NCH, 128, CSLICE).transpose(1, 0, 2)
            .reshape(128, NCH * CSLICE)).astype(BF)
        # spaced W_out rows: wo[32j+d, g, :] = W_out[c0 + 16*(4g+j) + d, :]
        wo = np.zeros((G, 4, 32, C), dtype=np.float32)
        wo[:, :, :HDIM, :] = W_out[c0:c0 + CSLICE, :].reshape(G, 4, HDIM, C)
        wo = np.ascontiguousarray(wo.reshape(G, 128, C).transpose(1, 0, 2))
        # bias row: attnT carries exact 1.0 at partition 16 (g=0, j=0 rowsum
        # row), so W_out row slot (g=0, row 16) adds b_out once (core 0).
        if p == 0:
            wo[16, 0, :] = b_out
        wo = np.ascontiguousarray(wo.reshape(128, G * C)).astype(BF)

        bq = np.zeros((G, 128), dtype=np.float32)
        bk = np.zeros((G, 128), dtype=np.float32)
        s2 = np.float32(1.0 / np.sqrt(2.0))
        for g in range(G):
            for j in range(4):
                h = HPC * p + 4 * g + j
                bq[g, 32 * j:32 * j + HDIM] = s2 * b_qkv[HDIM * h:HDIM * (h + 1)]
                bq[g, 32 * j + HDIM:32 * j + 32] = bq[g, 32 * j:32 * j + HDIM]
                bk[g, 32 * j:32 * j + HDIM] = s2 * b_qkv[C + HDIM * h:C + HDIM * (h + 1)]
                bk[g, 32 * j + HDIM:32 * j + 32] = bk[g, 32 * j:32 * j + HDIM]
        bv = np.ascontiguousarray(
            b_qkv[2 * C + c0:2 * C + c0 + CSLICE]).reshape(1, CSLICE)

        in_maps.append({
            "xt": xt, "wq": wq, "wk": wk, "wv": wv, "wo": wo,
            "bq": bq, "bk": bk, "bv": bv.astype(np.float32), "tri": tri,
        })
    return in_maps


def kernel(x, attn_mask, W_qkv, b_qkv, W_out, b_out):
    if "nc" not in _CACHE:
        nc = _build_nc()
        _legalize_waits(nc)   # sim-incompatible but required by walrus
        _CACHE["nc"] = nc
    nc = _CACHE["nc"]
    in_maps = _make_in_maps(x, W_qkv, b_qkv, W_out, b_out)
    res = run_bass_kernel_spmd(nc, in_maps, core_ids=list(range(NCORES)))
    y = np.zeros((T, C), dtype=np.float32)
    for r in res.results:
        y += r["y"].astype(np.float32)
    return y.reshape(1, T, C)


# revision 25
# speedup vs baseline: 1.0051x; 1.0051x over previous
"""Causal multi-head attention kernel for Trainium2 (8 NeuronCores), v3.

Problem: x[1,2048,1024] -> qkv proj (W_qkv[1024,3072]) -> 64 heads of dim 16
         -> causal softmax attention -> out proj (W_out[1024,1024]).

Sharding: Megatron-style head parallelism. Each of the 8 cores owns 8 heads
(a 128-wide column slice of W_qkv per q/k/v and a 128-row slice of W_out),
computes a partial output projection, and the host sums the 8 partials
(the "all-reduce").

v12 (~200us) changes vs v2 (316us baseline):
  * Host pre-transposes x into the [chan, chunk, token] bf16 layout and
    pre-spaces/pre-casts every weight to its SBUF layout: the entire 38us
    on-device staging prologue (fp32 loads + DVE casts + xbar transposes)
    becomes ~5MB of straight DMAs; matmuls start at ~3us.
  * PV matmuls are emitted one tile LATE (software-pipeline lag 1): the PE
    queue head never waits on an in-flight exp, so the PE stays dense and
    the HAM clock gate holds 2.4 GHz (v2 spent 189us at 1.2 GHz).
  * exp runs on BOTH ScalarE (table exp) and VectorE (Schraudolph bf16
    bit-trick, ~3% per-element error) with a tunable per-tile pattern,
    including diagonal tiles.
  * One full-width [128,512] pv*recip write per (group, qn) replaces four
    16-row strips; the softmax identity rowsum*recip==1 lands 1.0 in the
    spare partition rows, which doubles as the b_out bias row via an extra
    row in the pre-spaced W_out (bias comes free out of the out-proj MM).
  * Outproj evacuations alternate ACT/DVE; memsets moved to GpSimd; y is
    written bf16 (host sums partials in fp32).
  * outproj is deferred into the exp-bound late-qn phases (outproj(0)->qn2,
    outproj(1,2)->qn3 capped per group, outproj(3)->tail on the freed psco
    ring) so the PE never starves where the HAM clock gate would re-throttle.

  * HAM clock-gate feeding (the big one, ~227us -> ~200us): the HAM busy
    metric tracks PE array *switching activity*, not instruction occupancy —
    phases of pure S (16 live rows) / PV (17 live cols) matmuls re-throttle
    the PE to 1.2 GHz even with a dense instruction stream.  Fixes: q/k are
    duplicated across both 16-row halves of each 32-row group at 1/sqrt(2)
    (host prep) so S contracts 32 dense rows for the identical score; V's 15
    zero-pad columns carry duplicated v values (GpSimd copy) so PV/outproj
    run dense (the extra attnT rows meet zero W_out rows); dense qkv/v/
    outproj units are spread 1-per-2-tiles through the attention stream.

Known failed experiments (do not retry blindly): single-head [128,512] score
tiles with psco=4/ppv=2/pfil=2 (335us - instruction/sem count dominates);
all-ACT outproj evacs (278us - ACT head-of-line blocking); V computed as VT
with DMA-xbar transpose into the strided V layout (NaN - dma_start_transpose
requires the dst last dim to be the full in-partition extent); concentrating
outproj(1,2) into qn=3 (202us - qn=2 loses warmth; the even 1-per-2-tile
spread with outproj(s)->qn=s+1 is better).

Self-contained: hardcodes all shapes; host code only reshapes/slices inputs
per core and sums the 8 partial outputs.
"""

import numpy as np
from contextlib import ExitStack

import ml_dtypes

import concourse.bass as bass
import concourse.tile as tile
from concourse import mybir
from concourse.bass_utils import run_bass_kernel_spmd

F32 = mybir.dt.float32
BF16 = mybir.dt.bfloat16
I16 = mybir.dt.int16
AF = mybir.ActivationFunctionType

T = 2048
C = 1024
HDIM = 16
NHEADS = 64
NCORES = 8
HPC = NHEADS // NCORES      # 8 heads per core
CSLICE = HPC * HDIM         # 128 channel slice per core
G = 2                       # head groups of 4 per core
NCH = C // 128              # 8 contraction chunks
NT = T // 128               # 16 token chunks of 128
NQ = T // 512               # 4 query blocks of 512

# Schraudolph bf16 exp2: bits = round(EXPQ_MUL * s + EXPQ_ADD) viewed as bf16
# approximates exp(0.25*s).  128*log2(e)*0.25 = 46.166...; 16256 = 127<<7.
EXPQ_MUL = 128.0 * 0.25 * 1.4426950408889634
EXPQ_ADD = 16256.0 - 5.5

# Per-tile exp engine within each attention group: A=ScalarE exp table,
# D=VectorE Schraudolph trick.  Tile 0 of each group is pattern[0]; keep it
# 'A' so the first tile isn't stuck behind the previous group's DVE work.
EXP_PATTERN = "AD"
# PSUM evacuation engine alternation for qk/outproj units.
EVAC_PATTERN = "AD"

_CACHE = {}


def _legalize_waits(nc):
    """This neuronxcc/walrus build encodes at most ONE sync-wait per
    instruction (two on EventSemaphore) — multi-wait sync_info dies in
    codegen with "Too many sync wait commands".  Hoist excess waits into
    standalone EventSemaphore instructions on the same engine immediately
    before the instruction (engine queues are in-order, so semantics are
    preserved)."""
    import bass_rust
    n = 0
    for f in nc.m.functions:
        for blk in f.blocks:
            out = []
            changed = False
            for inst in blk.instructions:
                si = inst.sync_info
                waits = list(si.on_wait) if si is not None and si.on_wait else []
                cap = 2 if isinstance(inst, mybir.InstEventSemaphore) else 1
                if len(waits) > cap:
                    extra, keep = waits[:-cap], waits[-cap:]
                    for i in range(0, len(extra), 2):
                        ev = mybir.InstEventSemaphore(
                            name=f"evwait-{n}", ins=[], outs=[])
                        n += 1
                        ev.engine = inst.engine
                        ev.sync_info = bass_rust.SyncInfo(
                            on_wait=extra[i:i + 2], on_update=[])
                        out.append(ev)
                    inst.sync_info = bass_rust.SyncInfo(
                        on_wait=keep,
                        on_update=list(si.on_update) if si.on_update else [])
                    changed = True
                out.append(inst)
            if changed:
                blk.instructions = out
    return n


def _build_nc():
    nc = bass.Bass()

    xt_d = nc.declare_dram_parameter("xt", [128, NCH * T], BF16, isOutput=False)
    wq_d = nc.declare_dram_parameter("wq", [128, G * NCH * 128], BF16, isOutput=False)
    wk_d = nc.declare_dram_parameter("wk", [128, G * NCH * 128], BF16, isOutput=False)
    wv_d = nc.declare_dram_parameter("wv", [128, NCH * CSLICE], BF16, isOutput=False)
    wo_d = nc.declare_dram_parameter("wo", [128, G * C], BF16, isOutput=False)
    bq_d = nc.declare_dram_parameter("bq", [G, 128], F32, isOutput=False)
    bk_d = nc.declare_dram_parameter("bk", [G, 128], F32, isOutput=False)
    bv_d = nc.declare_dram_parameter("bv", [1, CSLICE], F32, isOutput=False)
    tri_d = nc.declare_dram_parameter("tri", [128, 128], BF16, isOutput=False)
    y_d = nc.declare_dram_parameter("y", [T, C], BF16, isOutput=True)

    with tile.TileContext(nc) as tc, ExitStack() as ctx:
        consts = ctx.enter_context(tc.tile_pool(name="consts", bufs=1))
        stage = ctx.enter_context(tc.tile_pool(name="stage", bufs=3))
        epool = ctx.enter_context(tc.tile_pool(name="epool", bufs=6))
        small = ctx.enter_context(tc.tile_pool(name="small", bufs=2))

        psco = ctx.enter_context(tc.tile_pool(name="psco", bufs=3, space="PSUM"))
        ppv = ctx.enter_context(tc.tile_pool(name="ppv", bufs=1, space="PSUM"))
        pfil = ctx.enter_context(tc.tile_pool(name="pfil", bufs=1, space="PSUM"))

        # ---- persistent tiles ----
        xT = consts.tile([128, NCH, T], BF16)   # xT[c, cc, t] = x[t, 128cc+c]
        qT = consts.tile([128, G, T], BF16)     # spaced: head j at part 32j
        kT = consts.tile([128, G, T], BF16)
        V = consts.tile([128, NT, HPC * 32], BF16)  # [t, tt, 8*32]: 16 dims +
        # rowsum-ones col + zero pad per head (packed PV writes 32 rows/head)
        vr = V.rearrange("p t (h e) -> p t h e", h=HPC)
        attnT = consts.tile([128, G, T], BF16)  # full 128 rows written by the
        # normalize TT: head rows = p/rowsum, spare rows = {1.0 (bias), 0}

        wq_sb = consts.tile([128, G, NCH, 128], BF16)
        wk_sb = consts.tile([128, G, NCH, 128], BF16)
        wv_sb = consts.tile([128, NCH, CSLICE], BF16)
        wo_sb = consts.tile([128, G, C], BF16)
        tri = consts.tile([128, 128], BF16)
        # pseudo-random bit patterns (iota int16 view, odd stride) give the
        # PE array high switching activity; dummy matmuls on it feed the HAM
        # clock-gate's busy metric (results land in a pfil bank, never read)
        scratch = consts.tile([128, 512], BF16)
        eps_sb = consts.tile([128, 1], F32)
        bq_sb = consts.tile([128, G], F32)
        bk_sb = consts.tile([128, G], F32)
        bv_sb = consts.tile([128, CSLICE], F32)

        # ---- input DMAs, two queues in parallel, first-needed first ----
        # sync queue: activations; scalar queue: weights/consts (ScalarE is
        # idle until the first exp anyway).
        xt_r = xt_d.rearrange("p (q a t) -> p q a t", q=NQ, a=NCH)
        nc.sync.dma_start(out=xT[:, 0:4, 0:512], in_=xt_r[:, 0, 0:4])
        nc.sync.dma_start(out=xT[:, 4:8, 0:512], in_=xt_r[:, 0, 4:8])
        wq_r = wq_d.rearrange("p (g a w) -> p g a w", g=G, a=NCH)
        wk_r = wk_d.rearrange("p (g a w) -> p g a w", g=G, a=NCH)
        nc.scalar.dma_start(out=wq_sb[:, 0], in_=wq_r[:, 0])
        nc.scalar.dma_start(out=wk_sb[:, 0], in_=wk_r[:, 0])
        nc.scalar.dma_start(out=tri, in_=tri_d[:, :])
        nc.scalar.dma_start(out=bq_sb, in_=bq_d.rearrange("g p -> p g"))
        nc.scalar.dma_start(out=bk_sb, in_=bk_d.rearrange("g p -> p g"))
        nc.scalar.dma_start(out=wq_sb[:, 1], in_=wq_r[:, 1])
        nc.scalar.dma_start(out=wk_sb[:, 1], in_=wk_r[:, 1])
        nc.scalar.dma_start(out=wv_sb, in_=wv_d.rearrange(
            "p (a w) -> p a w", a=NCH))
        nc.scalar.dma_start(out=bv_sb, in_=bv_d[0:1, :].to_broadcast((128, CSLICE)))
        nc.scalar.dma_start(out=wo_sb, in_=wo_d.rearrange("p (g w) -> p g w", g=G))

        nc.gpsimd.iota(scratch.bitcast(I16), [[197, 512]],
                       channel_multiplier=37,
                       allow_small_or_imprecise_dtypes=True)
        nc.vector.memset(eps_sb, 1e-30)
        # V pad columns: zeros at 17..31, rowsum-ones at 16 (GpSimd is idle)
        nc.gpsimd.memset(vr[:, :, :, HDIM:32], 0.0)
        nc.gpsimd.memset(vr[:, :, :, HDIM:HDIM + 1], 1.0)

        # ---- engine alternation counters ----
        exp_idx = [0]
        evac_idx = [0]

        def next_evac():
            e = EVAC_PATTERN[evac_idx[0] % len(EVAC_PATTERN)]
            evac_idx[0] += 1
            return e

        # ---- pipeline building blocks ----
        qk_open = {}

        def qk_half(g, qn, which, half):
            """One 4-chunk half of a q or k projection accumulation.
            half=0 opens the PSUM group; half=1 finishes it + bias evac."""
            w_sb, b_sb, dst = ((wq_sb, bq_sb, qT) if which == "q"
                               else (wk_sb, bk_sb, kT))
            if half == 0:
                qk_open[(which, g)] = pfil.tile(
                    [128, 512], F32, tag="fil", name="filps")
            ps_t = qk_open[(which, g)]
            for i in range(4):
                cc = 4 * half + i
                nc.tensor.matmul(
                    out=ps_t, lhsT=w_sb[:, g, cc, :],
                    rhs=xT[:, cc, qn * 512:(qn + 1) * 512],
                    start=(cc == 0), stop=(cc == NCH - 1),
                )
            if half == 1:
                del qk_open[(which, g)]
                nc.vector.tensor_scalar_add(
                    out=dst[:, g, qn * 512:(qn + 1) * 512], in0=ps_t,
                    scalar1=b_sb[:, g:g + 1])

        def v_tile(tt):
            ps_t = pfil.tile([128, 512], F32, tag="fil", name="vps")
            ps = ps_t[:, 0:CSLICE]
            for cc in range(NCH):
                nc.tensor.matmul(
                    out=ps, lhsT=xT[:, cc, tt * 128:(tt + 1) * 128],
                    rhs=wv_sb[:, cc, :],
                    start=(cc == 0), stop=(cc == NCH - 1),
                )
            nc.vector.tensor_tensor(
                vr[:, tt, :, 0:HDIM], ps.rearrange("p (h e) -> p h e", h=HPC),
                bv_sb.rearrange("p (h e) -> p h e", h=HPC),
                mybir.AluOpType.add,
            )
            # duplicate v into the 15 pad columns (GpSimd, off the critical
            # engines): PV then produces finite garbage in attnT rows whose
            # W_out rows are zero — results unchanged, but the PV and
            # outproj matmuls run with ~2x the PE array switching activity,
            # which keeps the HAM clock-gate busy metric fed.
            nc.gpsimd.tensor_copy(vr[:, tt, :, HDIM + 1:32],
                                  vr[:, tt, :, 1:HDIM])

        def outproj_unit(tt, nn, tail=False):
            if tail:
                # the attention psco ring is free at the tail: 3-deep
                # rotation instead of the single pfil buffer
                ps = psco.tile([128, 1024], F32, tag="sset", name="sset")[:, 0:512]
            else:
                ps = pfil.tile([128, 512], F32, tag="fil", name="ops")
            for g in range(G):
                nc.tensor.matmul(
                    out=ps, lhsT=attnT[:, g, tt * 128:(tt + 1) * 128],
                    rhs=wo_sb[:, g, nn * 512:(nn + 1) * 512],
                    start=(g == 0), stop=(g == G - 1),
                )
            ys = stage.tile([128, 512], BF16, tag="yout", name="ys")
            if next_evac() == "A":
                nc.scalar.activation(out=ys, in_=ps, func=AF.Copy)
            else:
                nc.vector.tensor_copy(ys, ps)
            nc.sync.dma_start(
                out=y_d[tt * 128:(tt + 1) * 128, nn * 512:(nn + 1) * 512],
                in_=ys,
            )

        def dummy_mms(k):
            ps = pfil.tile([128, 512], F32, tag="fil", name="dummy")
            for _ in range(k):
                nc.tensor.matmul(out=ps, lhsT=scratch[:, 0:128], rhs=scratch,
                                 start=True, stop=True)

        # filler scheduling: `fillers` (qkv/v — must drain within their qn so
        # consumers see the writes in emission order) popped 1/tile;
        # `late` (outproj — consumers are only the output DMAs) strided
        # across the exp-bound late-qn tile streams to keep the PE dense.
        fillers = []
        late = []

        def pop_any(k):
            for _ in range(k):
                if fillers:
                    fillers.pop(0)()
                elif late:
                    late.pop(0)()
                else:
                    return

        def attn_group(g, qn, late_cap=None):
            """Attention for 4 heads (group g) x 512 queries (block qn).
            Software-pipelined with PV lag 1: per step i we emit exp(i),
            PV(i-1), a filler, S(i+2) — the PE queue head never waits on an
            exp that was issued the same step, so the PE stream stays dense
            (keeps the HAM clock gate at 2.4 GHz)."""
            pv = ppv.tile([128, 512], F32, tag="pv")
            nkc = 4 * qn + 4
            tiles = [(kc, a) for kc in range(nkc) for a in range(2)]
            n = len(tiles)
            ssets = {}
            ets = {}

            def emit_S(i):
                kc, a = tiles[i]
                f0 = max(0, 128 * (kc - 4 * qn))
                sset = psco.tile([128, 1024], F32, tag="sset", name="sset")
                for jj in range(2):
                    j = 2 * a + jj
                    # q/k rows are duplicated across both 16-row halves at
                    # 1/sqrt(2) (host prep), so the 32-row contraction gives
                    # the same score with 2x the PE array switching activity
                    # (keeps the HAM clock gate's busy metric fed).
                    nc.tensor.matmul(
                        out=sset[:, 512 * jj + f0:512 * jj + 512],
                        lhsT=kT[32 * j:32 * j + 32, g, kc * 128:(kc + 1) * 128],
                        rhs=qT[32 * j:32 * j + 32, g, qn * 512 + f0:(qn + 1) * 512],
                        start=True, stop=True,
                        tile_position=(32 * j, 0),
                    )
                ssets[i] = sset

            def emit_exp(i):
                kc, a = tiles[i]
                jjj = kc - 4 * qn          # >=0: diagonal-straddling tile
                f0 = max(0, 128 * jjj)
                sset = ssets.pop(i)
                et = epool.tile([128, 1024], BF16, tag="expT", name="et")
                er = et.rearrange("p (h q) -> p h q", h=2)
                sr = sset.rearrange("p (h q) -> p h q", h=2)
                eng = EXP_PATTERN[exp_idx[0] % len(EXP_PATTERN)]
                exp_idx[0] += 1
                if eng == "D":
                    eb = et.bitcast(I16).rearrange("p (h q) -> p h q", h=2)
                    nc.vector.tensor_scalar(
                        out=eb[:, :, f0:512], in0=sr[:, :, f0:512],
                        scalar1=EXPQ_MUL, scalar2=EXPQ_ADD,
                        op0=mybir.AluOpType.mult,
                        op1=mybir.AluOpType.add,
                    )
                else:
                    nc.scalar.activation(
                        out=er[:, :, f0:512], in_=sr[:, :, f0:512],
                        func=AF.Exp, scale=0.25)
                if jjj >= 0:
                    # triangle-mask the diagonal stripe: on DVE for tricked
                    # tiles (bf16 2x mode, same queue as the trick -> no
                    # cross-engine hop), on GpSimd for ACT tiles.
                    if eng == "D":
                        nc.vector.tensor_tensor(
                            er[:, :, f0:f0 + 128], er[:, :, f0:f0 + 128],
                            tri[:, None, :].to_broadcast((128, 2, 128)),
                            mybir.AluOpType.mult,
                        )
                    else:
                        nc.gpsimd.tensor_tensor(
                            er[:, :, f0:f0 + 128], er[:, :, f0:f0 + 128],
                            tri[:, None, :].to_broadcast((128, 2, 128)),
                            mybir.AluOpType.mult,
                        )
                ets[i] = et

            def emit_PV(i):
                kc, a = tiles[i]
                f0 = max(0, 128 * (kc - 4 * qn))
                et = ets.pop(i)
                for jj in range(2):
                    j = 2 * a + jj
                    h = 4 * g + j
                    nc.tensor.matmul(
                        out=pv[32 * j:32 * j + 32, f0:512],
                        lhsT=V[:, kc, 32 * h:32 * h + 32],
                        rhs=et[:, 512 * jj + f0:512 * jj + 512],
                        start=(kc == 0), stop=(kc == nkc - 1),
                        tile_position=(0, 32 * j),
                        # sim group tracker is partition-base blind;
                        # packed heads write disjoint partitions
                        skip_group_check=True,
                    )

            exp_idx[0] = 0
            # spread the available late (outproj) fillers across this group's
            # tiles once the qkv/v fillers run dry; cap so later groups in
            # the same qn still get their share
            budget = len(late) if late_cap is None else min(late_cap, len(late))
            pops = 0
            emit_S(0)
            emit_S(1)
            for i in range(n):
                emit_exp(i)
                if i >= 1:
                    emit_PV(i - 1)
                if i % 2 == 0:
                    if fillers:
                        fillers.pop(0)()
                    elif late and pops < budget:
                        late.pop(0)()
                        pops += 1
                    elif qn == 3 and i % 4 == 2:
                        # no dense unit available: burn one dummy MM to hold
                        # the array-activity metric above the HAM threshold
                        dummy_mms(1)
                if i + 2 < n:
                    emit_S(i + 2)
            emit_PV(n - 1)
            # normalize: 1/rowsum via exp(-ln(x+eps)); garbage lanes may go
            # NaN/inf but only the (positive) rowsum lanes are ever read.
            ln_t = small.tile([128, 512], F32, tag="lnt")
            nc.scalar.activation(out=ln_t, in_=pv, func=AF.Ln, bias=eps_sb[:, 0:1])
            rec_t = small.tile([128, 512], F32, tag="rect")
            nc.scalar.activation(out=rec_t, in_=ln_t, func=AF.Exp, scale=-1.0)
            rec_rep = small.tile([128, 512], F32, tag="recrep")
            nc.vector.stream_shuffle(rec_rep, rec_t, [HDIM] * 32)
            # one full-width write: head rows p*recip, rowsum rows 1.0 (the
            # out-proj bias row), V-pad rows 0.
            nc.vector.tensor_tensor(
                attnT[:, g, qn * 512:(qn + 1) * 512], pv, rec_rep,
                mybir.AluOpType.mult,
            )
            # keep the PE fed while ln/exp/shuffle/mult run
            pop_any(3)

        # ---- emission: fused qn-major pipeline ----
        # qkv(qn=0): group 0 + v emitted directly, group 1 rides as filler
        # warm the HAM clock gate while the input DMAs land (~12us): the
        # first real matmuls then start at 2.4 GHz instead of cold-ramping
        dummy_mms(28)
        for half in range(2):
            qk_half(0, 0, "q", half)
        for half in range(2):
            qk_half(0, 0, "k", half)
        for tt in range(4):
            v_tile(tt)
        for half in range(2):
            fillers.append(lambda h=half: qk_half(1, 0, "q", h))
        for half in range(2):
            fillers.append(lambda h=half: qk_half(1, 0, "k", h))

        for qn in range(NQ):
            if qn + 1 < NQ:
                # next query block's activations (1MB, ~3us) — issued here so
                # the early sync queue stays clear for the V transposes
                nc.sync.dma_start(out=xT[:, :, (qn + 1) * 512:(qn + 2) * 512],
                                  in_=xt_r[:, qn + 1])
                for g in range(G):
                    for half in range(2):
                        fillers.append(
                            lambda g=g, qn=qn, h=half: qk_half(g, qn + 1, "q", h))
                    for half in range(2):
                        fillers.append(
                            lambda g=g, qn=qn, h=half: qk_half(g, qn + 1, "k", h))
                # v for qn+1 except the last block, whose tiles ride inside
                # qn=3 itself (its diagonal kc 12..15 aren't read until late
                # in each group, so emitting them early in qn=3 is safe and
                # keeps dense MMs flowing there)
                if qn < 2:
                    for tt in range(4 * qn + 4, 4 * qn + 8):
                        fillers.append(lambda tt=tt: v_tile(tt))
            if qn == 3:
                for tt in range(12, 16):
                    fillers.append(lambda tt=tt: v_tile(tt))
            # outproj is deferred one block (dense 128-contraction MMs keep
            # the PE array activity high where pure S/PV tiles would let the
            # HAM clock gate re-throttle): outproj(s) -> qn=s+1, 3 -> tail.
            if qn > 0:
                s = qn - 1
                for tt in range(4 * s, 4 * s + 4):
                    for nn in range(2):
                        late.append(lambda tt=tt, nn=nn: outproj_unit(tt, nn))
            for g in range(G):
                attn_group(g, qn, late_cap=(4 if g == 0 else None))
            # qkv/v fillers must be fully emitted before qn+1 consumes them
            while fillers:
                fillers.pop(0)()
        while late:
            late.pop(0)()
        for tt in range(12, 16):
            for nn in range(2):
                outproj_unit(tt, nn, tail=True)
    return nc


def _make_in_maps(x, W_qkv, b_qkv, W_out, b_out):
    BF = ml_dtypes.bfloat16
    x2 = np.asarray(x, dtype=np.float32).reshape(T, C)
    W_qkv = np.asarray(W_qkv, dtype=np.float32)
    b_qkv = np.asarray(b_qkv, dtype=np.float32)
    W_out = np.asarray(W_out, dtype=np.float32)
    b_out = np.asarray(b_out, dtype=np.float32)

    # chunk-major: xt[c, qn, cc, t'] = x[512qn + t', 128cc + c] so each
    # 512-token chunk DMA reads per-partition-contiguous 8KB runs
    xt = np.ascontiguousarray(
        x2.reshape(NQ, 512, NCH, 128).transpose(3, 0, 2, 1)).astype(BF)
    xt = np.ascontiguousarray(xt.reshape(128, NQ * NCH * 512))

    tri = np.zeros((128, 128), dtype=np.float32)
    for p in range(128):
        tri[p, p:] = 1.0
    tri = tri.astype(BF)

    in_maps = []
    for p in range(NCORES):
        c0 = p * CSLICE

        def spaced_w(cols0):
            # both 16-col halves of each head's 32-col slot carry w/sqrt(2):
            # the kernel contracts scores over the full 32-row group, giving
            # identical results with 2x the PE array switching activity
            blk = W_qkv[:, cols0:cols0 + CSLICE].reshape(NCH, 128, G, 4, HDIM)
            half = blk.transpose(1, 2, 0, 3, 4) * np.float32(1.0 / np.sqrt(2.0))
            sp = np.zeros((128, G, NCH, 4, 32), dtype=np.float32)
            sp[:, :, :, :, :HDIM] = half
            sp[:, :, :, :, HDIM:] = half
            return np.ascontiguousarray(
                sp.reshape(128, G * NCH * 128)).astype(BF)

        wq = spaced_w(c0)
        wk = spaced_w(C + c0)
        # wv[c, cc, e] = W_qkv[128cc+c, 2C + c0 + e]  (dense)
        wv = np.ascontiguousarray(
            W_qkv[:, 2 * C + c0:2 * C + c0 + CSLICE]
            .reshape(NCH, 128, CSLICE).transpose(1, 0, 2)
            .reshape(128, NCH * CSLICE)).astype(BF)
        # spaced W_out rows: wo[32j+d, g, :] = W_out[c0 + 16*(4g+j) + d, :]
        wo = np.zeros((G, 4, 32, C), dtype=np.float32)
        wo[:, :, :HDIM, :] = W_out[c0:c0 + CSLICE, :].reshape(G, 4, HDIM, C)
        wo = np.ascontiguousarray(wo.reshape(G, 128, C).transpose(1, 0, 2))
        # bias row: attnT carries exact 1.0 at partition 16 (g=0, j=0 rowsum
        # row), so W_out row slot (g=0, row 16) adds b_out once (core 0).
        if p == 0:
            wo[16, 0, :] = b_out
        wo = np.ascontiguousarray(wo.reshape(128, G * C)).astype(BF)

        bq = np.zeros((G, 128), dtype=np.float32)
        bk = np.zeros((G, 128), dtype=np.float32)
        s2 = np.float32(1.0 / np.sqrt(2.0))
        for g in range(G):
            for j in range(4):
                h = HPC * p + 4 * g + j
                bq[g, 32 * j:32 * j + HDIM] = s2 * b_qkv[HDIM * h:HDIM * (h + 1)]
                bq[g, 32 * j + HDIM:32 * j + 32] = bq[g, 32 * j:32 * j + HDIM]
                bk[g, 32 * j:32 * j + HDIM] = s2 * b_qkv[C + HDIM * h:C + HDIM * (h + 1)]
                bk[g, 32 * j + HDIM:32 * j + 32] = bk[g, 32 * j:32 * j + HDIM]
        bv = np.ascontiguousarray(
            b_qkv[2 * C + c0:2 * C + c0 + CSLICE]).reshape(1, CSLICE)

        in_maps.append({
            "xt": xt, "wq": wq, "wk": wk, "wv": wv, "wo": wo,
            "bq": bq, "bk": bk, "bv": bv.astype(np.float32), "tri": tri,
        })
    return in_maps


def kernel(x, attn_mask, W_qkv, b_qkv, W_out, b_out):
    if "nc" not in _CACHE:
        nc = _build_nc()
        _legalize_waits(nc)   # sim-incompatible but required by walrus
        _CACHE["nc"] = nc
    nc = _CACHE["nc"]
    in_maps = _make_in_maps(x, W_qkv, b_qkv, W_out, b_out)
    res = run_bass_kernel_spmd(nc, in_maps, core_ids=list(range(NCORES)))
    y = np.zeros((T, C), dtype=np.float32)
    for r in res.results:
        y += r["y"].astype(np.float32)
    return y.reshape(1, T, C)


# revision 26
# speedup vs baseline: 1.0569x; 1.0515x over previous
"""Causal multi-head attention kernel for Trainium2 (8 NeuronCores), v3.

Problem: x[1,2048,1024] -> qkv proj (W_qkv[1024,3072]) -> 64 heads of dim 16
         -> causal softmax attention -> out proj (W_out[1024,1024]).

Sharding: Megatron-style head parallelism. Each of the 8 cores owns 8 heads
(a 128-wide column slice of W_qkv per q/k/v and a 128-row slice of W_out),
computes a partial output projection, and the host sums the 8 partials
(the "all-reduce").

v12 (~200us) changes vs v2 (316us baseline):
  * Host pre-transposes x into the [chan, chunk, token] bf16 layout and
    pre-spaces/pre-casts every weight to its SBUF layout: the entire 38us
    on-device staging prologue (fp32 loads + DVE casts + xbar transposes)
    becomes ~5MB of straight DMAs; matmuls start at ~3us.
  * PV matmuls are emitted one tile LATE (software-pipeline lag 1): the PE
    queue head never waits on an in-flight exp, so the PE stays dense and
    the HAM clock gate holds 2.4 GHz (v2 spent 189us at 1.2 GHz).
  * exp runs on BOTH ScalarE (table exp) and VectorE (Schraudolph bf16
    bit-trick, ~3% per-element error) with a tunable per-tile pattern,
    including diagonal tiles.
  * One full-width [128,512] pv*recip write per (group, qn) replaces four
    16-row strips; the softmax identity rowsum*recip==1 lands 1.0 in the
    spare partition rows, which doubles as the b_out bias row via an extra
    row in the pre-spaced W_out (bias comes free out of the out-proj MM).
  * Outproj evacuations alternate ACT/DVE; memsets moved to GpSimd; y is
    written bf16 (host sums partials in fp32).
  * outproj is deferred into the exp-bound late-qn phases (outproj(0)->qn2,
    outproj(1,2)->qn3 capped per group, outproj(3)->tail on the freed psco
    ring) so the PE never starves where the HAM clock gate would re-throttle.

  * HAM clock-gate feeding (the big one, ~227us -> ~200us): the HAM busy
    metric tracks PE array *switching activity*, not instruction occupancy —
    phases of pure S (16 live rows) / PV (17 live cols) matmuls re-throttle
    the PE to 1.2 GHz even with a dense instruction stream.  Fixes: q/k are
    duplicated across both 16-row halves of each 32-row group at 1/sqrt(2)
    (host prep) so S contracts 32 dense rows for the identical score; V's 15
    zero-pad columns carry duplicated v values (GpSimd copy) so PV/outproj
    run dense (the extra attnT rows meet zero W_out rows); dense qkv/v/
    outproj units are spread 1-per-2-tiles through the attention stream.

Known failed experiments (do not retry blindly): single-head [128,512] score
tiles with psco=4/ppv=2/pfil=2 (335us - instruction/sem count dominates);
all-ACT outproj evacs (278us - ACT head-of-line blocking); V computed as VT
with DMA-xbar transpose into the strided V layout (NaN - dma_start_transpose
requires the dst last dim to be the full in-partition extent); concentrating
outproj(1,2) into qn=3 (202us - qn=2 loses warmth; the even 1-per-2-tile
spread with outproj(s)->qn=s+1 is better).

Self-contained: hardcodes all shapes; host code only reshapes/slices inputs
per core and sums the 8 partial outputs.
"""

import numpy as np
from contextlib import ExitStack

import ml_dtypes

import concourse.bass as bass
import concourse.tile as tile
from concourse import mybir
from concourse.bass_utils import run_bass_kernel_spmd

F32 = mybir.dt.float32
BF16 = mybir.dt.bfloat16
I16 = mybir.dt.int16
AF = mybir.ActivationFunctionType

T = 2048
C = 1024
HDIM = 16
NHEADS = 64
NCORES = 8
HPC = NHEADS // NCORES      # 8 heads per core
CSLICE = HPC * HDIM         # 128 channel slice per core
G = 2                       # head groups of 4 per core
NCH = C // 128              # 8 contraction chunks
NT = T // 128               # 16 token chunks of 128
NQ = T // 512               # 4 query blocks of 512

# Schraudolph bf16 exp2: bits = round(EXPQ_MUL * s + EXPQ_ADD) viewed as bf16
# approximates exp(0.25*s).  128*log2(e)*0.25 = 46.166...; 16256 = 127<<7.
EXPQ_MUL = 128.0 * 0.25 * 1.4426950408889634
EXPQ_ADD = 16256.0 - 5.5

# Per-tile exp engine within each attention group: A=ScalarE exp table,
# D=VectorE Schraudolph trick.  Tile 0 of each group is pattern[0]; keep it
# 'A' so the first tile isn't stuck behind the previous group's DVE work.
EXP_PATTERN = "AD"
# PSUM evacuation engine alternation for qk/outproj units.
EVAC_PATTERN = "AD"

_CACHE = {}


def _legalize_waits(nc):
    """This neuronxcc/walrus build encodes at most ONE sync-wait per
    instruction (two on EventSemaphore) — multi-wait sync_info dies in
    codegen with "Too many sync wait commands".  Hoist excess waits into
    standalone EventSemaphore instructions on the same engine immediately
    before the instruction (engine queues are in-order, so semantics are
    preserved)."""
    import bass_rust
    n = 0
    for f in nc.m.functions:
        for blk in f.blocks:
            out = []
            changed = False
            for inst in blk.instructions:
                si = inst.sync_info
                waits = list(si.on_wait) if si is not None and si.on_wait else []
                cap = 2 if isinstance(inst, mybir.InstEventSemaphore) else 1
                if len(waits) > cap:
                    extra, keep = waits[:-cap], waits[-cap:]
                    for i in range(0, len(extra), 2):
                        ev = mybir.InstEventSemaphore(
                            name=f"evwait-{n}", ins=[], outs=[])
                        n += 1
                        ev.engine = inst.engine
                        ev.sync_info = bass_rust.SyncInfo(
                            on_wait=extra[i:i + 2], on_update=[])
                        out.append(ev)
                    inst.sync_info = bass_rust.SyncInfo(
                        on_wait=keep,
                        on_update=list(si.on_update) if si.on_update else [])
                    changed = True
                out.append(inst)
            if changed:
                blk.instructions = out
    return n


def _build_nc():
    nc = bass.Bass()

    xt_d = nc.declare_dram_parameter("xt", [128, NCH * T], BF16, isOutput=False)
    wq_d = nc.declare_dram_parameter("wq", [128, G * NCH * 128], BF16, isOutput=False)
    wk_d = nc.declare_dram_parameter("wk", [128, G * NCH * 128], BF16, isOutput=False)
    wv_d = nc.declare_dram_parameter("wv", [128, NCH * CSLICE], BF16, isOutput=False)
    wo_d = nc.declare_dram_parameter("wo", [128, G * C], BF16, isOutput=False)
    bq_d = nc.declare_dram_parameter("bq", [G, 128], F32, isOutput=False)
    bk_d = nc.declare_dram_parameter("bk", [G, 128], F32, isOutput=False)
    bv_d = nc.declare_dram_parameter("bv", [1, CSLICE], F32, isOutput=False)
    tri_d = nc.declare_dram_parameter("tri", [128, 128], BF16, isOutput=False)
    y_d = nc.declare_dram_parameter("y", [T, C], BF16, isOutput=True)

    with tile.TileContext(nc) as tc, ExitStack() as ctx:
        consts = ctx.enter_context(tc.tile_pool(name="consts", bufs=1))
        stage = ctx.enter_context(tc.tile_pool(name="stage", bufs=3))
        epool = ctx.enter_context(tc.tile_pool(name="epool", bufs=6))
        small = ctx.enter_context(tc.tile_pool(name="small", bufs=2))

        psco = ctx.enter_context(tc.tile_pool(name="psco", bufs=3, space="PSUM"))
        ppv = ctx.enter_context(tc.tile_pool(name="ppv", bufs=1, space="PSUM"))
        pfil = ctx.enter_context(tc.tile_pool(name="pfil", bufs=1, space="PSUM"))

        # ---- persistent tiles ----
        xT = consts.tile([128, NCH, T], BF16)   # xT[c, cc, t] = x[t, 128cc+c]
        qT = consts.tile([128, G, T], BF16)     # spaced: head j at part 32j
        kT = consts.tile([128, G, T], BF16)
        V = consts.tile([128, NT, HPC * 32], BF16)  # [t, tt, 8*32]: 16 dims +
        # rowsum-ones col + zero pad per head (packed PV writes 32 rows/head)
        vr = V.rearrange("p t (h e) -> p t h e", h=HPC)
        attnT = consts.tile([128, G, T], BF16)  # full 128 rows written by the
        # normalize TT: head rows = p/rowsum, spare rows = {1.0 (bias), 0}

        wq_sb = consts.tile([128, G, NCH, 128], BF16)
        wk_sb = consts.tile([128, G, NCH, 128], BF16)
        wv_sb = consts.tile([128, NCH, CSLICE], BF16)
        wo_sb = consts.tile([128, G, C], BF16)
        tri = consts.tile([128, 128], BF16)
        # pseudo-random bit patterns (iota int16 view, odd stride) give the
        # PE array high switching activity; dummy matmuls on it feed the HAM
        # clock-gate's busy metric (results land in a pfil bank, never read)
        scratch = consts.tile([128, 512], BF16)
        eps_sb = consts.tile([128, 1], F32)
        bq_sb = consts.tile([128, G], F32)
        bk_sb = consts.tile([128, G], F32)
        bv_sb = consts.tile([128, CSLICE], F32)

        # ---- input DMAs, two queues in parallel, first-needed first ----
        # sync queue: activations; scalar queue: weights/consts (ScalarE is
        # idle until the first exp anyway).
        xt_r = xt_d.rearrange("p (q a t) -> p q a t", q=NQ, a=NCH)
        nc.sync.dma_start(out=xT[:, 0:4, 0:512], in_=xt_r[:, 0, 0:4])
        nc.sync.dma_start(out=xT[:, 4:8, 0:512], in_=xt_r[:, 0, 4:8])
        wq_r = wq_d.rearrange("p (g a w) -> p g a w", g=G, a=NCH)
        wk_r = wk_d.rearrange("p (g a w) -> p g a w", g=G, a=NCH)
        nc.scalar.dma_start(out=wq_sb[:, 0], in_=wq_r[:, 0])
        nc.scalar.dma_start(out=wk_sb[:, 0], in_=wk_r[:, 0])
        nc.scalar.dma_start(out=tri, in_=tri_d[:, :])
        nc.scalar.dma_start(out=bq_sb, in_=bq_d.rearrange("g p -> p g"))
        nc.scalar.dma_start(out=bk_sb, in_=bk_d.rearrange("g p -> p g"))
        nc.scalar.dma_start(out=wq_sb[:, 1], in_=wq_r[:, 1])
        nc.scalar.dma_start(out=wk_sb[:, 1], in_=wk_r[:, 1])
        nc.scalar.dma_start(out=wv_sb, in_=wv_d.rearrange(
            "p (a w) -> p a w", a=NCH))
        nc.scalar.dma_start(out=bv_sb, in_=bv_d[0:1, :].to_broadcast((128, CSLICE)))
        nc.scalar.dma_start(out=wo_sb, in_=wo_d.rearrange("p (g w) -> p g w", g=G))

        nc.gpsimd.iota(scratch.bitcast(I16), [[197, 512]],
                       channel_multiplier=37,
                       allow_small_or_imprecise_dtypes=True)
        nc.vector.memset(eps_sb, 1e-30)
        # V pad columns: zeros at 17..31, rowsum-ones at 16 (GpSimd is idle)
        nc.gpsimd.memset(vr[:, :, :, HDIM:32], 0.0)
        nc.gpsimd.memset(vr[:, :, :, HDIM:HDIM + 1], 1.0)

        # ---- engine alternation counters ----
        exp_idx = [0]
        evac_idx = [0]

        def next_evac():
            e = EVAC_PATTERN[evac_idx[0] % len(EVAC_PATTERN)]
            evac_idx[0] += 1
            return e

        # ---- pipeline building blocks ----
        qk_open = {}

        def qk_half(g, qn, which, half):
            """One 4-chunk half of a q or k projection accumulation.
            half=0 opens the PSUM group; half=1 finishes it + bias evac."""
            w_sb, b_sb, dst = ((wq_sb, bq_sb, qT) if which == "q"
                               else (wk_sb, bk_sb, kT))
            if half == 0:
                qk_open[(which, g)] = pfil.tile(
                    [128, 512], F32, tag="fil", name="filps")
            ps_t = qk_open[(which, g)]
            for i in range(4):
                cc = 4 * half + i
                nc.tensor.matmul(
                    out=ps_t, lhsT=w_sb[:, g, cc, :],
                    rhs=xT[:, cc, qn * 512:(qn + 1) * 512],
                    start=(cc == 0), stop=(cc == NCH - 1),
                )
            if half == 1:
                del qk_open[(which, g)]
                nc.vector.tensor_scalar_add(
                    out=dst[:, g, qn * 512:(qn + 1) * 512], in0=ps_t,
                    scalar1=b_sb[:, g:g + 1])

        def v_tile(tt):
            ps_t = pfil.tile([128, 512], F32, tag="fil", name="vps")
            ps = ps_t[:, 0:CSLICE]
            for cc in range(NCH):
                nc.tensor.matmul(
                    out=ps, lhsT=xT[:, cc, tt * 128:(tt + 1) * 128],
                    rhs=wv_sb[:, cc, :],
                    start=(cc == 0), stop=(cc == NCH - 1),
                )
            nc.vector.tensor_tensor(
                vr[:, tt, :, 0:HDIM], ps.rearrange("p (h e) -> p h e", h=HPC),
                bv_sb.rearrange("p (h e) -> p h e", h=HPC),
                mybir.AluOpType.add,
            )
            # duplicate v into the 15 pad columns (GpSimd, off the critical
            # engines): PV then produces finite garbage in attnT rows whose
            # W_out rows are zero — results unchanged, but the PV and
            # outproj matmuls run with ~2x the PE array switching activity,
            # which keeps the HAM clock-gate busy metric fed.
            nc.gpsimd.tensor_copy(vr[:, tt, :, HDIM + 1:32],
                                  vr[:, tt, :, 1:HDIM])

        def outproj_unit(tt, nn, tail=False):
            if tail:
                # the attention psco ring is free at the tail: 3-deep
                # rotation instead of the single pfil buffer
                ps = psco.tile([128, 1024], F32, tag="sset", name="sset")[:, 0:512]
            else:
                ps = pfil.tile([128, 512], F32, tag="fil", name="ops")
            for g in range(G):
                nc.tensor.matmul(
                    out=ps, lhsT=attnT[:, g, tt * 128:(tt + 1) * 128],
                    rhs=wo_sb[:, g, nn * 512:(nn + 1) * 512],
                    start=(g == 0), stop=(g == G - 1),
                )
            ys = stage.tile([128, 512], BF16, tag="yout", name="ys")
            if next_evac() == "A":
                nc.scalar.activation(out=ys, in_=ps, func=AF.Copy)
            else:
                nc.vector.tensor_copy(ys, ps)
            nc.sync.dma_start(
                out=y_d[tt * 128:(tt + 1) * 128, nn * 512:(nn + 1) * 512],
                in_=ys,
            )

        def dummy_mms(k):
            ps = pfil.tile([128, 512], F32, tag="fil", name="dummy")
            for _ in range(k):
                nc.tensor.matmul(out=ps, lhsT=scratch[:, 0:128], rhs=scratch,
                                 start=True, stop=True)

        # filler scheduling: `fillers` (qkv/v — must drain within their qn so
        # consumers see the writes in emission order) popped 1/tile;
        # `late` (outproj — consumers are only the output DMAs) strided
        # across the exp-bound late-qn tile streams to keep the PE dense.
        fillers = []
        late = []

        def pop_any(k):
            for _ in range(k):
                if fillers:
                    fillers.pop(0)()
                elif late:
                    late.pop(0)()
                else:
                    return

        def attn_group(g, qn, late_cap=None):
            """Attention for 4 heads (group g) x 512 queries (block qn).
            Software-pipelined with PV lag 1: per step i we emit exp(i),
            PV(i-1), a filler, S(i+2) — the PE queue head never waits on an
            exp that was issued the same step, so the PE stream stays dense
            (keeps the HAM clock gate at 2.4 GHz)."""
            pv = ppv.tile([128, 512], F32, tag="pv")
            nkc = 4 * qn + 4
            tiles = [(kc, a) for kc in range(nkc) for a in range(2)]
            n = len(tiles)
            ssets = {}
            ets = {}

            def emit_S(i):
                kc, a = tiles[i]
                f0 = max(0, 128 * (kc - 4 * qn))
                sset = psco.tile([128, 1024], F32, tag="sset", name="sset")
                for jj in range(2):
                    j = 2 * a + jj
                    # q/k rows are duplicated across both 16-row halves at
                    # 1/sqrt(2) (host prep), so the 32-row contraction gives
                    # the same score with 2x the PE array switching activity
                    # (keeps the HAM clock gate's busy metric fed).
                    nc.tensor.matmul(
                        out=sset[:, 512 * jj + f0:512 * jj + 512],
                        lhsT=kT[32 * j:32 * j + 32, g, kc * 128:(kc + 1) * 128],
                        rhs=qT[32 * j:32 * j + 32, g, qn * 512 + f0:(qn + 1) * 512],
                        start=True, stop=True,
                        tile_position=(32 * j, 0),
                    )
                ssets[i] = sset

            def emit_exp(i):
                kc, a = tiles[i]
                jjj = kc - 4 * qn          # >=0: diagonal-straddling tile
                f0 = max(0, 128 * jjj)
                sset = ssets.pop(i)
                et = epool.tile([128, 1024], BF16, tag="expT", name="et")
                er = et.rearrange("p (h q) -> p h q", h=2)
                sr = sset.rearrange("p (h q) -> p h q", h=2)
                eng = EXP_PATTERN[exp_idx[0] % len(EXP_PATTERN)]
                exp_idx[0] += 1
                if eng == "D":
                    eb = et.bitcast(I16).rearrange("p (h q) -> p h q", h=2)
                    nc.vector.tensor_scalar(
                        out=eb[:, :, f0:512], in0=sr[:, :, f0:512],
                        scalar1=EXPQ_MUL, scalar2=EXPQ_ADD,
                        op0=mybir.AluOpType.mult,
                        op1=mybir.AluOpType.add,
                    )
                else:
                    nc.scalar.activation(
                        out=er[:, :, f0:512], in_=sr[:, :, f0:512],
                        func=AF.Exp, scale=0.25)
                if jjj >= 0:
                    # triangle-mask the diagonal stripe: on DVE for tricked
                    # tiles (bf16 2x mode, same queue as the trick -> no
                    # cross-engine hop), on GpSimd for ACT tiles.
                    if eng == "D":
                        nc.vector.tensor_tensor(
                            er[:, :, f0:f0 + 128], er[:, :, f0:f0 + 128],
                            tri[:, None, :].to_broadcast((128, 2, 128)),
                            mybir.AluOpType.mult,
                        )
                    else:
                        nc.gpsimd.tensor_tensor(
                            er[:, :, f0:f0 + 128], er[:, :, f0:f0 + 128],
                            tri[:, None, :].to_broadcast((128, 2, 128)),
                            mybir.AluOpType.mult,
                        )
                ets[i] = et

            def emit_PV(i):
                kc, a = tiles[i]
                f0 = max(0, 128 * (kc - 4 * qn))
                et = ets.pop(i)
                for jj in range(2):
                    j = 2 * a + jj
                    h = 4 * g + j
                    nc.tensor.matmul(
                        out=pv[32 * j:32 * j + 32, f0:512],
                        lhsT=V[:, kc, 32 * h:32 * h + 32],
                        rhs=et[:, 512 * jj + f0:512 * jj + 512],
                        start=(kc == 0), stop=(kc == nkc - 1),
                        tile_position=(0, 32 * j),
                        # sim group tracker is partition-base blind;
                        # packed heads write disjoint partitions
                        skip_group_check=True,
                    )

            exp_idx[0] = 0
            # spread the available late (outproj) fillers across this group's
            # tiles once the qkv/v fillers run dry; cap so later groups in
            # the same qn still get their share
            budget = len(late) if late_cap is None else min(late_cap, len(late))
            pops = 0
            emit_S(0)
            emit_S(1)
            for i in range(n):
                emit_exp(i)
                if i >= 1:
                    emit_PV(i - 1)
                if i % 2 == 0:
                    if fillers:
                        fillers.pop(0)()
                    elif late and pops < budget:
                        late.pop(0)()
                        pops += 1
                if i + 2 < n:
                    emit_S(i + 2)
            emit_PV(n - 1)
            # normalize: 1/rowsum via exp(-ln(x+eps)); garbage lanes may go
            # NaN/inf but only the (positive) rowsum lanes are ever read.
            ln_t = small.tile([128, 512], F32, tag="lnt")
            nc.scalar.activation(out=ln_t, in_=pv, func=AF.Ln, bias=eps_sb[:, 0:1])
            rec_t = small.tile([128, 512], F32, tag="rect")
            nc.scalar.activation(out=rec_t, in_=ln_t, func=AF.Exp, scale=-1.0)
            rec_rep = small.tile([128, 512], F32, tag="recrep")
            nc.vector.stream_shuffle(rec_rep, rec_t, [HDIM] * 32)
            # one full-width write: head rows p*recip, rowsum rows 1.0 (the
            # out-proj bias row), V-pad rows 0.
            nc.vector.tensor_tensor(
                attnT[:, g, qn * 512:(qn + 1) * 512], pv, rec_rep,
                mybir.AluOpType.mult,
            )
            # keep the PE fed while ln/exp/shuffle/mult run
            pop_any(3)

        # ---- emission: fused qn-major pipeline ----
        # qkv(qn=0): group 0 + v emitted directly, group 1 rides as filler
        # warm the HAM clock gate while the input DMAs land (~12us): the
        # first real matmuls then start at 2.4 GHz instead of cold-ramping
        dummy_mms(28)
        for half in range(2):
            qk_half(0, 0, "q", half)
        for half in range(2):
            qk_half(0, 0, "k", half)
        for tt in range(4):
            v_tile(tt)
        for half in range(2):
            fillers.append(lambda h=half: qk_half(1, 0, "q", h))
        for half in range(2):
            fillers.append(lambda h=half: qk_half(1, 0, "k", h))

        for qn in range(NQ):
            if qn + 1 < NQ:
                # next query block's activations (1MB, ~3us) — issued here so
                # the early sync queue stays clear for the V transposes
                nc.sync.dma_start(out=xT[:, :, (qn + 1) * 512:(qn + 2) * 512],
                                  in_=xt_r[:, qn + 1])
                for g in range(G):
                    for half in range(2):
                        fillers.append(
                            lambda g=g, qn=qn, h=half: qk_half(g, qn + 1, "q", h))
                    for half in range(2):
                        fillers.append(
                            lambda g=g, qn=qn, h=half: qk_half(g, qn + 1, "k", h))
                # v for qn+1 except the last block, whose tiles ride inside
                # qn=3 itself (its diagonal kc 12..15 aren't read until late
                # in each group, so emitting them early in qn=3 is safe and
                # keeps dense MMs flowing there)
                if qn < 2:
                    for tt in range(4 * qn + 4, 4 * qn + 8):
                        fillers.append(lambda tt=tt: v_tile(tt))
            if qn == 3:
                for tt in range(12, 16):
                    fillers.append(lambda tt=tt: v_tile(tt))
            # outproj is deferred one block (dense 128-contraction MMs keep
            # the PE array activity high where pure S/PV tiles would let the
            # HAM clock gate re-throttle): outproj(s) -> qn=s+1, 3 -> tail.
            if qn > 0:
                s = qn - 1
                for tt in range(4 * s, 4 * s + 4):
                    for nn in range(2):
                        late.append(lambda tt=tt, nn=nn: outproj_unit(tt, nn))
            for g in range(G):
                attn_group(g, qn, late_cap=(4 if g == 0 else None))
            # qkv/v fillers must be fully emitted before qn+1 consumes them
            while fillers:
                fillers.pop(0)()
        while late:
            late.pop(0)()
        for tt in range(12, 16):
            for nn in range(2):
                outproj_unit(tt, nn, tail=True)
    return nc


def _make_in_maps(x, W_qkv, b_qkv, W_out, b_out):
    BF = ml_dtypes.bfloat16
    x2 = np.asarray(x, dtype=np.float32).reshape(T, C)
    W_qkv = np.asarray(W_qkv, dtype=np.float32)
    b_qkv = np.asarray(b_qkv, dtype=np.float32)
    W_out = np.asarray(W_out, dtype=np.float32)
    b_out = np.asarray(b_out, dtype=np.float32)

    # chunk-major: xt[c, qn, cc, t'] = x[512qn + t', 128cc + c] so each
    # 512-token chunk DMA reads per-partition-contiguous 8KB runs
    xt = np.ascontiguousarray(
        x2.reshape(NQ, 512, NCH, 128).transpose(3, 0, 2, 1)).astype(BF)
    xt = np.ascontiguousarray(xt.reshape(128, NQ * NCH * 512))

    tri = np.zeros((128, 128), dtype=np.float32)
    for p in range(128):
        tri[p, p:] = 1.0
    tri = tri.astype(BF)

    in_maps = []
    for p in range(NCORES):
        c0 = p * CSLICE

        def spaced_w(cols0):
            # both 16-col halves of each head's 32-col slot carry w/sqrt(2):
            # the kernel contracts scores over the full 32-row group, giving
            # identical results with 2x the PE array switching activity
            blk = W_qkv[:, cols0:cols0 + CSLICE].reshape(NCH, 128, G, 4, HDIM)
            half = blk.transpose(1, 2, 0, 3, 4) * np.float32(1.0 / np.sqrt(2.0))
            sp = np.zeros((128, G, NCH, 4, 32), dtype=np.float32)
            sp[:, :, :, :, :HDIM] = half
            sp[:, :, :, :, HDIM:] = half
            return np.ascontiguousarray(
                sp.reshape(128, G * NCH * 128)).astype(BF)

        wq = spaced_w(c0)
        wk = spaced_w(C + c0)
        # wv[c, cc, e] = W_qkv[128cc+c, 2C + c0 + e]  (dense)
        wv = np.ascontiguousarray(
            W_qkv[:, 2 * C + c0:2 * C + c0 + CSLICE]
            .reshape(NCH, 128, CSLICE).transpose(1, 0, 2)
            .reshape(128, NCH * CSLICE)).astype(BF)
        # spaced W_out rows: wo[32j+d, g, :] = W_out[c0 + 16*(4g+j) + d, :]
        wo = np.zeros((G, 4, 32, C), dtype=np.float32)
        wo[:, :, :HDIM, :] = W_out[c0:c0 + CSLICE, :].reshape(G, 4, HDIM, C)
        wo = np.ascontiguousarray(wo.reshape(G, 128, C).transpose(1, 0, 2))
        # bias row: attnT carries exact 1.0 at partition 16 (g=0, j=0 rowsum
        # row), so W_out row slot (g=0, row 16) adds b_out once (core 0).
        if p == 0:
            wo[16, 0, :] = b_out
        wo = np.ascontiguousarray(wo.reshape(128, G * C)).astype(BF)

        bq = np.zeros((G, 128), dtype=np.float32)
        bk = np.zeros((G, 128), dtype=np.float32)
        s2 = np.float32(1.0 / np.sqrt(2.0))
        for g in range(G):
            for j in range(4):
                h = HPC * p + 4 * g + j
                bq[g, 32 * j:32 * j + HDIM] = s2 * b_qkv[HDIM * h:HDIM * (h + 1)]
                bq[g, 32 * j + HDIM:32 * j + 32] = bq[g, 32 * j:32 * j + HDIM]
                bk[g, 32 * j:32 * j + HDIM] = s2 * b_qkv[C + HDIM * h:C + HDIM * (h + 1)]
                bk[g, 32 * j + HDIM:32 * j + 32] = bk[g, 32 * j:32 * j + HDIM]
        bv = np.ascontiguousarray(
            b_qkv[2 * C + c0:2 * C + c0 + CSLICE]).reshape(1, CSLICE)

        in_maps.append({
            "xt": xt, "wq": wq, "wk": wk, "wv": wv, "wo": wo,
            "bq": bq, "bk": bk, "bv": bv.astype(np.float32), "tri": tri,
        })
    return in_maps


def kernel(x, attn_mask, W_qkv, b_qkv, W_out, b_out):
    if "nc" not in _CACHE:
        nc = _build_nc()
        _legalize_waits(nc)   # sim-incompatible but required by walrus
        _CACHE["nc"] = nc
    nc = _CACHE["nc"]
    in_maps = _make_in_maps(x, W_qkv, b_qkv, W_out, b_out)
    res = run_bass_kernel_spmd(nc, in_maps, core_ids=list(range(NCORES)))
    y = np.zeros((T, C), dtype=np.float32)
    for r in res.results:
        y += r["y"].astype(np.float32)
    return y.reshape(1, T, C)
